# revision 19
# baseline (speedup 1.0000x reference)
"""CoarseToFine gather+proj+merge kernel for 8 Trainium2 NeuronCores.

Reference computation (per match i of M, for two branches):
  window = 5x5 patch of fine map (stride-4 grid, pad 2), flattened
           CHANNEL-major then re-read as [25, 128] (torch-unfold + plain
           reshape => "scrambled" (c,k)->(a,d) relabeling)
  bias   = coarse[b, l] @ Wcomb.T + bcomb          (folded proj+merge1)
  out    = window_scrambled @ Wmerge2.T + bias     -> [25, 128]

Sharding: items (2 branches x M) are partitioned by (branch, b, h-half)
into 8 groups, one per core.  Each core receives the 121-row HWC slice
of the one padded fine map its windows touch, the matching coarse map,
and host-built int16 gather row indices.  All compute (gathers, the
scramble, matmuls, bias, transposes) runs on-device.

Device pipeline per core:
  dma_gather fine window-rows (5px x 128ch = 2560B each, 4px-aligned)
  -> PE kj-block transposes => per-item channel-major [128c, 25k] tiles
  -> DRAM bounce (write c-major raster, read back as [25,128] rows)
     == the reference's reshape
  -> PE transpose -> [128d, cols] -> matmul Wm2 -> +bias (coarse path:
     dma_gather + PE transposes + folded-weight matmuls)
  -> PE transpose to [item, 25, 128] -> DMA out.
"""

import os
import numpy as np

WINDOW = 5
C = 128        # fine channels
H, W = 240, 320
HP, WP = 244, 324          # padded fine map dims (pad 2 each side)
HO, WO = 60, 80            # coarse grid
L = 4800                   # coarse positions
DC = 256                   # coarse dim
ROWS = 121                 # padded rows per half-map slice
B = 2
IC = 16                    # items per compute chunk (16*25 = 400 cols)
TB = 100                   # out-transpose block cols (4 per chunk)
GC = 128                   # items per gather chunk (640 window-rows)
NBLK = 9800                # 4px-aligned gather rows in the map slice


# --------------------------------------------------------------------------
# sync-wait legalization: this walrus build accepts only ONE sync wait per
# instruction; overflow waits move to NOPs inserted just before, same engine.
def _split_sync_waits(nc, mybir, max_waits=1):
    for fn in nc.m.functions:
        for blk in fn.blocks:
            new_insts = []
            for inst in blk.instructions:
                si = getattr(inst, "sync_info", None)
                waits = list(si.on_wait) if si is not None and si.on_wait else []
                if len(waits) > max_waits:
                    for wt in waits[:-max_waits]:
                        nop = mybir.InstNoOp(
                            name=nc.get_next_instruction_name(),
                            engine=inst.engine,
                            ins=[],
                            outs=[],
                            sync_info=mybir.SyncInfo(on_wait=[wt], on_update=[]),
                        )
                        nc.register_instruction(nop)
                        new_insts.append(nop)
                    si.on_wait = waits[-max_waits:]
                new_insts.append(inst)
            blk.instructions = new_insts
    return nc


# --------------------------------------------------------------------------
def _build_program(CAP):
    import concourse.bass as bass
    import concourse.bacc as bacc
    import concourse.mybir as mybir
    import concourse.tile as tile
    from concourse.masks import make_identity

    CAPG = CAP // 128          # coarse gather slots / gather chunks
    NCHUNK = CAP // IC         # compute chunks
    dt = mybir.dt

    nc = bacc.Bacc("TRN2", target_bir_lowering=False, debug=False, num_devices=8)

    fmap = nc.dram_tensor("fmap", [1, ROWS * WP * C], dt.float32, kind="ExternalInput").ap()
    cfeat = nc.dram_tensor("cfeat", [1, L * DC], dt.float32, kind="ExternalInput").ap()
    fidx = nc.dram_tensor("fidx", [128, CAP * 5 // 16], dt.int16, kind="ExternalInput").ap()
    cidx = nc.dram_tensor("cidx", [128, CAP // 16], dt.int16, kind="ExternalInput").ap()
    wproj = nc.dram_tensor("wproj", [128, 256], dt.float32, kind="ExternalInput").ap()
    wmerge = nc.dram_tensor("wmerge", [128, 256], dt.float32, kind="ExternalInput").ap()
    bproj = nc.dram_tensor("bproj", [128], dt.float32, kind="ExternalInput").ap()
    bmerge = nc.dram_tensor("bmerge", [128], dt.float32, kind="ExternalInput").ap()
    out = nc.dram_tensor("out", [128 * CAP * 25], dt.float32, kind="ExternalOutput").ap()

    fine_src = bass.AP(fmap.tensor, 0, [[512, NBLK], [1, 640]])
    coarse_src = bass.AP(cfeat.tensor, 0, [[256, L], [1, 256]])

    with tile.TileContext(nc) as tc:
        with (
            tc.tile_pool(name="const", bufs=1) as cpool,
            tc.tile_pool(name="t2", bufs=2) as t2pool,
            tc.tile_pool(name="xs", bufs=2) as xspool,
            tc.tile_pool(name="tsb", bufs=2) as tpool,
            tc.tile_pool(name="merged", bufs=2) as mpool,
        ):
            ident = cpool.tile([128, 128], dt.float32)
            make_identity(nc, ident)

            wp_sb = cpool.tile([128, 256], dt.float32)
            wm_sb = cpool.tile([128, 256], dt.float32)
            bp_sb = cpool.tile([128, 1], dt.float32)
            bm_sb = cpool.tile([128, 1], dt.float32)
            nc.sync.dma_start(wp_sb[:], wproj[:])
            nc.sync.dma_start(wm_sb[:], wmerge[:])
            nc.sync.dma_start(bp_sb[:], bproj[:].unsqueeze(1))
            nc.sync.dma_start(bm_sb[:], bmerge[:].unsqueeze(1))

            fidx_sb = cpool.tile([128, CAP * 5 // 16], dt.int16)
            cidx_sb = cpool.tile([128, CAP // 16], dt.int16)
            nc.sync.dma_start(fidx_sb[:], fidx[:])
            nc.sync.dma_start(cidx_sb[:], cidx[:])

            wm1t = cpool.tile([128, 128], dt.float32)
            wm2t = cpool.tile([128, 128], dt.float32)
            wctA = cpool.tile([128, 128], dt.float32)
            wctB = cpool.tile([128, 128], dt.float32)
            bcomb = cpool.tile([128, 1], dt.float32)
            ct0 = cpool.tile([128, CAP], dt.float32)
            ct1 = cpool.tile([128, CAP], dt.float32)
            bias_sb = cpool.tile([128, CAP], dt.float32)
            cc_sb = cpool.tile([128, CAPG * 256], dt.float32)

            with tc.tile_pool(name="psprep", bufs=2, space="PSUM") as psw:
                # folded weights: wm1t = Wmerge[:, :128].T ; wm2t = Wmerge[:, 128:].T
                tps = psw.tile([128, 128], dt.float32, space="PSUM", tag="w")
                nc.tensor.transpose(tps[:], wm_sb[:, 0:128], ident[:])
                nc.vector.tensor_copy(wm1t[:], tps[:])
                tps2 = psw.tile([128, 128], dt.float32, space="PSUM", tag="w")
                nc.tensor.transpose(tps2[:], wm_sb[:, 128:256], ident[:])
                nc.vector.tensor_copy(wm2t[:], tps2[:])

                # WcombT chunks: wct{A,B}[k, o] = sum_j Wproj[j, kchunk] * Wm1[o, j]
                wps = psw.tile([128, 128], dt.float32, space="PSUM", tag="w")
                nc.tensor.matmul(wps[:], lhsT=wp_sb[:, 0:128], rhs=wm1t[:], start=True, stop=True)
                nc.vector.tensor_copy(wctA[:], wps[:])
                wps2 = psw.tile([128, 128], dt.float32, space="PSUM", tag="w")
                nc.tensor.matmul(wps2[:], lhsT=wp_sb[:, 128:256], rhs=wm1t[:], start=True, stop=True)
                nc.vector.tensor_copy(wctB[:], wps2[:])

                # bcomb[o] = Wm1 @ b_proj + b_merge  (as [128, 1] column)
                bps = psw.tile([128, 1], dt.float32, space="PSUM", tag="w")
                nc.tensor.matmul(bps[:], lhsT=wm1t[:], rhs=bp_sb[:], start=True, stop=True)
                nc.vector.tensor_add(bcomb[:], bps[:], bm_sb[:])

                # coarse branch: gather rows (item j -> [j%128, j//128]),
                # transpose to [k, item], project to per-item bias columns
                nc.gpsimd.dma_gather(
                    out_ap=cc_sb[:].rearrange("p (g d) -> p g d", d=256),
                    in_ap=coarse_src,
                    idxs_ap=cidx_sb[:],
                    num_idxs=CAP,
                    num_idxs_reg=CAP,
                    elem_size=256,
                )
                for t in range(CAPG):
                    for kc, ct in ((0, ct0), (1, ct1)):
                        cps = psw.tile([128, 128], dt.float32, space="PSUM", tag="w")
                        nc.tensor.transpose(
                            cps[:], cc_sb[:, t * 256 + kc * 128: t * 256 + (kc + 1) * 128],
                            ident[:],
                        )
                        nc.vector.tensor_copy(ct[:, t * 128:(t + 1) * 128], cps[:])

                for t in range(CAPG):
                    bmm = psw.tile([128, 128], dt.float32, space="PSUM", tag="w")
                    nc.tensor.matmul(bmm[:], lhsT=wctA[:], rhs=ct0[:, t * 128:(t + 1) * 128],
                                     start=True, stop=False)
                    nc.tensor.matmul(bmm[:], lhsT=wctB[:], rhs=ct1[:, t * 128:(t + 1) * 128],
                                     start=False, stop=True)
                    nc.vector.tensor_scalar_add(bias_sb[:, t * 128:(t + 1) * 128],
                                                bmm[:], bcomb[:])

            # ---- fine branch
            # Host idx order places window-row (item m, ki) at gf partition
            # m, slot kc*5+ki, so per partition gf holds the item's own
            # window, pixel-major (ki, kj, c).  The reference's scramble is
            # then a pure free-dim permutation to channel-major (c, ki, kj):
            # one strided copy per chunk.  Stage 2 PE-transposes 128-wide
            # q-blocks to put the contraction dim on partitions.
            gfa = cpool.tile([128, CAPG * 5 * 640], dt.float32, tag="gf")
            nc.gpsimd.dma_gather(
                out_ap=gfa[:].rearrange("p (g d) -> p g d", d=640),
                in_ap=fine_src,
                idxs_ap=fidx_sb[:],
                num_idxs=CAP * 5,
                num_idxs_reg=CAP * 5,
                elem_size=640,
                elem_step=512,
                single_packet=False,
            )
            with (
                tc.tile_pool(name="pstp", bufs=2, space="PSUM") as pstp,
                tc.tile_pool(name="psmm", bufs=2, space="PSUM") as psmm,
            ):
                gfv = gfa[:].rearrange("p (kc ki kj c) -> p kc ki kj c",
                                       kc=CAPG, ki=5, kj=5)
                for kc in range(CAPG):          # chunk: 128 items
                    t3 = t2pool.tile([128, GC * 25], dt.float32, tag="t3")
                    nc.vector.tensor_copy(
                        t3[:].rearrange("m (c ki kj) -> m ki kj c", ki=5, kj=5),
                        gfv[:, kc],
                    )
                    # stage 2: per q-block transpose -> tsb[d, m*25+a]
                    tsb = tpool.tile([128, GC * 25], dt.float32, tag="ts")
                    tsv = tsb[:].rearrange("p (m a) -> p m a", a=25)
                    for ag in range(7):         # groups of <=4 a-blocks
                        a0 = ag * 4
                        na = min(4, 25 - a0)
                        tp = pstp.tile([128, 512], dt.float32, space="PSUM", tag="tp")
                        for ai in range(na):
                            nc.tensor.transpose(
                                tp[:, ai * 128:(ai + 1) * 128],
                                t3[:, (a0 + ai) * 128:(a0 + ai + 1) * 128], ident[:])
                        nc.vector.tensor_copy(
                            tsv[:, :, a0:a0 + na],
                            tp[:, :na * 128].rearrange("p (a m) -> p m a", a=na),
                        )

                    merged = mpool.tile([128, GC * 25], dt.float32, tag="mg")
                    for kl in range(GC // IC):  # compute chunk: 16 items
                        k = kc * (GC // IC) + kl
                        mm = psmm.tile([128, IC * 25], dt.float32, space="PSUM", tag="mm")
                        nc.tensor.matmul(mm[:], lhsT=wm2t[:],
                                         rhs=tsb[:, kl * IC * 25:(kl + 1) * IC * 25],
                                         start=True, stop=True)
                        nc.vector.tensor_add(
                            merged[:, kl * IC * 25:(kl + 1) * IC * 25]
                            .rearrange("p (i w) -> p i w", w=25),
                            mm[:].rearrange("p (i w) -> p i w", w=25),
                            bias_sb[:, k * IC:(k + 1) * IC].unsqueeze(2).broadcast_to([128, IC, 25]),
                        )
                    nc.sync.dma_start(
                        out.rearrange("(o q) -> o q", o=128)[:, kc * GC * 25:(kc + 1) * GC * 25],
                        merged[:],
                    )

    nc.compile()
    _split_sync_waits(nc, mybir)
    return nc


# --------------------------------------------------------------------------
def _wrap16(vals, ncols):
    """int16 index layout for dma_gather: idx j at [j%16, j//16], replicated
    to all 8 Q7 core groups (partitions 16g+p)."""
    w = np.zeros((16, ncols), np.int16)
    w[np.arange(len(vals)) % 16, np.arange(len(vals)) // 16] = vals
    return np.tile(w, (8, 1))


def _host_prep(inputs):
    f0 = np.asarray(inputs["feat_f0"], np.float32)
    f1 = np.asarray(inputs["feat_f1"], np.float32)
    c0 = np.asarray(inputs["feat_c0"], np.float32)
    c1 = np.asarray(inputs["feat_c1"], np.float32)
    b_ids = np.asarray(inputs["b_ids"]).astype(np.int64)
    l_ids = np.asarray(inputs["l_ids"]).astype(np.int64)
    s_ids = np.asarray(inputs["s_ids"]).astype(np.int64)
    wproj = np.asarray(inputs["W_proj"], np.float32)
    bproj = np.asarray(inputs["b_proj"], np.float32)
    wmerge = np.asarray(inputs["W_merge"], np.float32)
    bmerge = np.asarray(inputs["b_merge"], np.float32)
    M = b_ids.shape[0]

    # pad + HWC layout: [B, HP, WP, C]
    fpadT = [
        np.ascontiguousarray(
            np.pad(f, ((0, 0), (0, 0), (2, 2), (2, 2))).transpose(0, 2, 3, 1))
        for f in (f0, f1)
    ]

    groups = []  # (positions_into_2M, ids, branch, b, half)
    for branch, ids in ((0, l_ids), (1, s_ids)):
        h = ids // WO
        for bb in range(B):
            for half in range(2):
                mask = (b_ids == bb) & ((h >= 30) if half else (h < 30))
                pos = np.nonzero(mask)[0] + branch * M
                groups.append((pos, ids[mask], branch, bb, half))

    maxcnt = max(len(g[1]) for g in groups)
    CAP = max(((maxcnt + 127) // 128) * 128, 128)

    in_maps = []
    for pos, ids, branch, bb, half in groups:
        rs = 120 if half else 0
        fm = fpadT[branch][bb, rs:rs + ROWS]               # [121, 324, 128]
        cf = (c0, c1)[branch][bb]                          # [4800, 256]

        idp = np.zeros(CAP, np.int64)
        idp[:len(ids)] = ids
        if len(ids) < CAP:
            idp[len(ids):] = 0 if half == 0 else 30 * WO
        h = idp // WO
        w = idp % WO
        # window-row gather blocks: row (4h - rs + r), col block w (4px units).
        # dma_gather places row j at [j%128, j//128]; order rows so item m
        # (chunk-local) lands on partition m with its 5 ki rows in slots 0-4:
        # position (within chunk) = ki*128 + m_local.
        blk = ((4 * h - rs)[:, None] + np.arange(5)[None, :]) * (WP // 4) + w[:, None]
        blk = blk.reshape(-1, GC, 5).transpose(0, 2, 1)   # [chunk, ki, m_local]
        fidx = _wrap16(blk.reshape(-1).astype(np.int16), CAP * 5 // 16)
        cidx = _wrap16(idp.astype(np.int16), CAP // 16)

        in_maps.append({
            "fmap": np.ascontiguousarray(fm).reshape(1, -1),
            "cfeat": np.ascontiguousarray(cf).reshape(1, -1),
            "fidx": fidx,
            "cidx": cidx,
            "wproj": wproj,
            "wmerge": wmerge,
            "bproj": bproj,
            "bmerge": bmerge,
        })
    return in_maps, groups, CAP, M


def _assemble(results, groups, M):
    full = np.empty((2 * M, 25, 128), np.float32)
    for (pos, ids, *_), res in zip(groups, results):
        og = res["out"].reshape(128, -1, 25).transpose(1, 2, 0)
        full[pos] = og[:len(pos)]
    return full[:M], full[M:]


def _install_ntff_shim():
    """This image lacks ``antenv.axon_hooks``; recreate it so bass_utils'
    trace path can drive NTFF profiling via the axon PJRT .so."""
    import sys, types
    if "antenv.axon_hooks" in sys.modules:
        return
    import antenv  # noqa: F401
    mod = types.ModuleType("antenv.axon_hooks")
    mod._hook = None
    mod.set_axon_ntff_profile_hook = lambda h: setattr(mod, "_hook", h)
    mod.get_axon_ntff_profile_hook = lambda: mod._hook
    sys.modules["antenv.axon_hooks"] = mod
    try:
        from trn_agent_boot.trn_boot import _ntff_profile_via_ctypes
        mod._hook = _ntff_profile_via_ctypes("/opt/axon/libaxon_pjrt.so")
    except Exception:
        pass


def kernel(**inputs):
    from concourse import bass_utils

    in_maps, groups, CAP, M = _host_prep(inputs)
    nc = _build_program(CAP)

    if os.environ.get("TRNK_SIM"):
        from concourse.bass_interp import CoreSim
        results = []
        ncore = int(os.environ.get("TRNK_SIM_CORES", "8"))
        for c in range(8):
            if c < ncore:
                sim = CoreSim(nc, trace=False)
                for name, val in in_maps[c].items():
                    sim.tensor(name)[:] = val
                sim.simulate()
                results.append({"out": np.array(sim.tensor("out"))})
            else:
                results.append({"out": np.zeros(CAP * 3200, np.float32)})
        return _assemble(results, groups, M)

    trace = bool(os.environ.get("TRNK_TRACE"))
    kw = {}
    if trace:
        _install_ntff_shim()
        kw = dict(trace=True, trace_cores=list(range(8)))
    res = bass_utils.run_bass_kernel_spmd(nc, in_maps, core_ids=list(range(8)), **kw)
    if trace and res.exec_time_ns is not None:
        kernel.last_exec_time_ns = res.exec_time_ns
        kernel.last_mean_exec_time_ns = res.mean_exec_time_ns
        if res.instructions_and_trace:
            kernel.last_trace_path = res.instructions_and_trace[1]
    return _assemble(res.results, groups, M)


kernel.last_exec_time_ns = None
kernel.last_mean_exec_time_ns = None
kernel.last_trace_path = None


# revision 20
# speedup vs baseline: 1.2804x; 1.2804x over previous
"""CoarseToFine gather+proj+merge kernel for 8 Trainium2 NeuronCores.

Reference computation (per match i of M, for two branches):
  window = 5x5 patch of fine map (stride-4 grid, pad 2), flattened
           CHANNEL-major then re-read as [25, 128] (torch-unfold + plain
           reshape => "scrambled" (c,k)->(a,d) relabeling)
  bias   = coarse[b, l] @ Wcomb.T + bcomb          (folded proj+merge1)
  out    = window_scrambled @ Wmerge2.T + bias     -> [25, 128]

Sharding: items (2 branches x M) are partitioned by (branch, b, h-half)
into 8 groups, one per core.  Each core receives the 121-row HWC slice
of the one padded fine map its windows touch, the matching coarse map,
and host-built int16 gather row indices.  All compute (gathers, the
scramble, matmuls, bias, transposes) runs on-device.

Device pipeline per core:
  dma_gather fine window-rows (5px x 128ch = 2560B each, 4px-aligned)
  -> PE kj-block transposes => per-item channel-major [128c, 25k] tiles
  -> DRAM bounce (write c-major raster, read back as [25,128] rows)
     == the reference's reshape
  -> PE transpose -> [128d, cols] -> matmul Wm2 -> +bias (coarse path:
     dma_gather + PE transposes + folded-weight matmuls)
  -> PE transpose to [item, 25, 128] -> DMA out.
"""

import os
import numpy as np

WINDOW = 5
C = 128        # fine channels
H, W = 240, 320
HP, WP = 244, 324          # padded fine map dims (pad 2 each side)
HO, WO = 60, 80            # coarse grid
L = 4800                   # coarse positions
DC = 256                   # coarse dim
ROWS = 121                 # padded rows per half-map slice
B = 2
IC = 16                    # items per compute chunk (16*25 = 400 cols)
TB = 100                   # out-transpose block cols (4 per chunk)
GC = 128                   # items per gather chunk (640 window-rows)
NBLK = 9800                # 4px-aligned gather rows in the map slice


# --------------------------------------------------------------------------
# sync-wait legalization: this walrus build accepts only ONE sync wait per
# instruction; overflow waits move to NOPs inserted just before, same engine.
def _split_sync_waits(nc, mybir, max_waits=1):
    for fn in nc.m.functions:
        for blk in fn.blocks:
            new_insts = []
            for inst in blk.instructions:
                si = getattr(inst, "sync_info", None)
                waits = list(si.on_wait) if si is not None and si.on_wait else []
                if len(waits) > max_waits:
                    for wt in waits[:-max_waits]:
                        nop = mybir.InstNoOp(
                            name=nc.get_next_instruction_name(),
                            engine=inst.engine,
                            ins=[],
                            outs=[],
                            sync_info=mybir.SyncInfo(on_wait=[wt], on_update=[]),
                        )
                        nc.register_instruction(nop)
                        new_insts.append(nop)
                    si.on_wait = waits[-max_waits:]
                new_insts.append(inst)
            blk.instructions = new_insts
    return nc


# --------------------------------------------------------------------------
def _build_program(CAP):
    import concourse.bass as bass
    import concourse.bacc as bacc
    import concourse.mybir as mybir
    import concourse.tile as tile
    from concourse.masks import make_identity

    CAPG = CAP // 128          # coarse gather slots / gather chunks
    NCHUNK = CAP // IC         # compute chunks
    dt = mybir.dt

    nc = bacc.Bacc("TRN2", target_bir_lowering=False, debug=False, num_devices=8)

    fmap = nc.dram_tensor("fmap", [1, ROWS * WP * C], dt.float32, kind="ExternalInput").ap()
    cfeat = nc.dram_tensor("cfeat", [1, L * DC], dt.float32, kind="ExternalInput").ap()
    fidx = nc.dram_tensor("fidx", [128, CAP * 5 // 16], dt.int16, kind="ExternalInput").ap()
    cidx = nc.dram_tensor("cidx", [128, CAP // 16], dt.int16, kind="ExternalInput").ap()
    wproj = nc.dram_tensor("wproj", [128, 256], dt.float32, kind="ExternalInput").ap()
    wmerge = nc.dram_tensor("wmerge", [128, 256], dt.float32, kind="ExternalInput").ap()
    bproj = nc.dram_tensor("bproj", [128], dt.float32, kind="ExternalInput").ap()
    bmerge = nc.dram_tensor("bmerge", [128], dt.float32, kind="ExternalInput").ap()
    out = nc.dram_tensor("out", [128 * CAP * 25], dt.float32, kind="ExternalOutput").ap()

    fine_src = bass.AP(fmap.tensor, 0, [[512, NBLK], [1, 640]])
    coarse_src = bass.AP(cfeat.tensor, 0, [[256, L], [1, 256]])

    with tile.TileContext(nc) as tc:
        with (
            tc.tile_pool(name="const", bufs=1) as cpool,
            tc.tile_pool(name="gf", bufs=3) as gfpool,
            tc.tile_pool(name="t2", bufs=2) as t2pool,
            tc.tile_pool(name="xs", bufs=2) as xspool,
            tc.tile_pool(name="tsb", bufs=2) as tpool,
            tc.tile_pool(name="merged", bufs=2) as mpool,
        ):
            ident = cpool.tile([128, 128], dt.float32)
            make_identity(nc, ident)

            wp_sb = cpool.tile([128, 256], dt.float32)
            wm_sb = cpool.tile([128, 256], dt.float32)
            bp_sb = cpool.tile([128, 1], dt.float32)
            bm_sb = cpool.tile([128, 1], dt.float32)
            nc.sync.dma_start(wp_sb[:], wproj[:])
            nc.sync.dma_start(wm_sb[:], wmerge[:])
            nc.sync.dma_start(bp_sb[:], bproj[:].unsqueeze(1))
            nc.sync.dma_start(bm_sb[:], bmerge[:].unsqueeze(1))

            fidx_sb = cpool.tile([128, CAP * 5 // 16], dt.int16)
            cidx_sb = cpool.tile([128, CAP // 16], dt.int16)
            nc.sync.dma_start(fidx_sb[:], fidx[:])
            nc.sync.dma_start(cidx_sb[:], cidx[:])

            wm1t = cpool.tile([128, 128], dt.float32)
            wm2t = cpool.tile([128, 128], dt.float32)
            wctA = cpool.tile([128, 128], dt.float32)
            wctB = cpool.tile([128, 128], dt.float32)
            bcomb = cpool.tile([128, 1], dt.float32)
            ct0 = cpool.tile([128, CAP], dt.float32)
            ct1 = cpool.tile([128, CAP], dt.float32)
            bias_sb = cpool.tile([128, CAP], dt.float32)
            cc_sb = cpool.tile([128, CAPG * 256], dt.float32)

            with tc.tile_pool(name="psprep", bufs=2, space="PSUM") as psw:
                # folded weights: wm1t = Wmerge[:, :128].T ; wm2t = Wmerge[:, 128:].T
                tps = psw.tile([128, 128], dt.float32, space="PSUM", tag="w")
                nc.tensor.transpose(tps[:], wm_sb[:, 0:128], ident[:])
                nc.vector.tensor_copy(wm1t[:], tps[:])
                tps2 = psw.tile([128, 128], dt.float32, space="PSUM", tag="w")
                nc.tensor.transpose(tps2[:], wm_sb[:, 128:256], ident[:])
                nc.vector.tensor_copy(wm2t[:], tps2[:])

                # WcombT chunks: wct{A,B}[k, o] = sum_j Wproj[j, kchunk] * Wm1[o, j]
                wps = psw.tile([128, 128], dt.float32, space="PSUM", tag="w")
                nc.tensor.matmul(wps[:], lhsT=wp_sb[:, 0:128], rhs=wm1t[:], start=True, stop=True)
                nc.vector.tensor_copy(wctA[:], wps[:])
                wps2 = psw.tile([128, 128], dt.float32, space="PSUM", tag="w")
                nc.tensor.matmul(wps2[:], lhsT=wp_sb[:, 128:256], rhs=wm1t[:], start=True, stop=True)
                nc.vector.tensor_copy(wctB[:], wps2[:])

                # bcomb[o] = Wm1 @ b_proj + b_merge  (as [128, 1] column)
                bps = psw.tile([128, 1], dt.float32, space="PSUM", tag="w")
                nc.tensor.matmul(bps[:], lhsT=wm1t[:], rhs=bp_sb[:], start=True, stop=True)
                nc.vector.tensor_add(bcomb[:], bps[:], bm_sb[:])

                # coarse branch: gather rows (item j -> [j%128, j//128]),
                # transpose to [k, item], project to per-item bias columns
                nc.gpsimd.dma_gather(
                    out_ap=cc_sb[:].rearrange("p (g d) -> p g d", d=256),
                    in_ap=coarse_src,
                    idxs_ap=cidx_sb[:],
                    num_idxs=CAP,
                    num_idxs_reg=CAP,
                    elem_size=256,
                )
                for t in range(CAPG):
                    for kc, ct in ((0, ct0), (1, ct1)):
                        cps = psw.tile([128, 128], dt.float32, space="PSUM", tag="w")
                        nc.tensor.transpose(
                            cps[:], cc_sb[:, t * 256 + kc * 128: t * 256 + (kc + 1) * 128],
                            ident[:],
                        )
                        nc.vector.tensor_copy(ct[:, t * 128:(t + 1) * 128], cps[:])

                for t in range(CAPG):
                    bmm = psw.tile([128, 128], dt.float32, space="PSUM", tag="w")
                    nc.tensor.matmul(bmm[:], lhsT=wctA[:], rhs=ct0[:, t * 128:(t + 1) * 128],
                                     start=True, stop=False)
                    nc.tensor.matmul(bmm[:], lhsT=wctB[:], rhs=ct1[:, t * 128:(t + 1) * 128],
                                     start=False, stop=True)
                    nc.vector.tensor_scalar_add(bias_sb[:, t * 128:(t + 1) * 128],
                                                bmm[:], bcomb[:])

            # ---- fine branch
            # Host idx order places window-row (item m, ki) at gf partition
            # m, slot kc*5+ki, so per partition gf holds the item's own
            # window, pixel-major (ki, kj, c).  The reference's scramble is
            # then a pure free-dim permutation to channel-major (c, ki, kj):
            # one strided copy per chunk.  Stage 2 PE-transposes 128-wide
            # q-blocks to put the contraction dim on partitions.
            with (
                tc.tile_pool(name="pstp", bufs=2, space="PSUM") as pstp,
                tc.tile_pool(name="psmm", bufs=2, space="PSUM") as psmm,
            ):
                for kc in range(CAPG):          # chunk: 128 items
                    gf = gfpool.tile([128, 5 * 640], dt.float32, tag="gf")
                    nc.gpsimd.dma_gather(
                        out_ap=gf[:].rearrange("p (g d) -> p g d", d=640),
                        in_ap=fine_src,
                        idxs_ap=fidx_sb[:, kc * 40:(kc + 1) * 40],
                        num_idxs=640,
                        num_idxs_reg=640,
                        elem_size=640,
                        elem_step=512,
                    )
                    t3 = t2pool.tile([128, GC * 25], dt.float32, tag="t3")
                    nc.vector.tensor_copy(
                        t3[:].rearrange("m (c ki kj) -> m ki kj c", ki=5, kj=5),
                        gf[:].rearrange("m (ki kj c) -> m ki kj c", ki=5, kj=5),
                    )
                    # stage 2: per q-block transpose -> tsb[d, m*25+a]
                    tsb = tpool.tile([128, GC * 25], dt.float32, tag="ts")
                    tsv = tsb[:].rearrange("p (m a) -> p m a", a=25)
                    for ag in range(7):         # groups of <=4 a-blocks
                        a0 = ag * 4
                        na = min(4, 25 - a0)
                        tp = pstp.tile([128, 512], dt.float32, space="PSUM", tag="tp")
                        for ai in range(na):
                            nc.tensor.transpose(
                                tp[:, ai * 128:(ai + 1) * 128],
                                t3[:, (a0 + ai) * 128:(a0 + ai + 1) * 128], ident[:])
                        nc.vector.tensor_copy(
                            tsv[:, :, a0:a0 + na],
                            tp[:, :na * 128].rearrange("p (a m) -> p m a", a=na),
                        )

                    merged = mpool.tile([128, GC * 25], dt.float32, tag="mg")
                    for kl in range(GC // IC):  # compute chunk: 16 items
                        k = kc * (GC // IC) + kl
                        mm = psmm.tile([128, IC * 25], dt.float32, space="PSUM", tag="mm")
                        nc.tensor.matmul(mm[:], lhsT=wm2t[:],
                                         rhs=tsb[:, kl * IC * 25:(kl + 1) * IC * 25],
                                         start=True, stop=True)
                        nc.vector.tensor_add(
                            merged[:, kl * IC * 25:(kl + 1) * IC * 25]
                            .rearrange("p (i w) -> p i w", w=25),
                            mm[:].rearrange("p (i w) -> p i w", w=25),
                            bias_sb[:, k * IC:(k + 1) * IC].unsqueeze(2).broadcast_to([128, IC, 25]),
                        )
                    nc.sync.dma_start(
                        out.rearrange("(o q) -> o q", o=128)[:, kc * GC * 25:(kc + 1) * GC * 25],
                        merged[:],
                    )

    nc.compile()
    _split_sync_waits(nc, mybir)
    return nc


# --------------------------------------------------------------------------
def _wrap16(vals, ncols):
    """int16 index layout for dma_gather: idx j at [j%16, j//16], replicated
    to all 8 Q7 core groups (partitions 16g+p)."""
    w = np.zeros((16, ncols), np.int16)
    w[np.arange(len(vals)) % 16, np.arange(len(vals)) // 16] = vals
    return np.tile(w, (8, 1))


def _host_prep(inputs):
    f0 = np.asarray(inputs["feat_f0"], np.float32)
    f1 = np.asarray(inputs["feat_f1"], np.float32)
    c0 = np.asarray(inputs["feat_c0"], np.float32)
    c1 = np.asarray(inputs["feat_c1"], np.float32)
    b_ids = np.asarray(inputs["b_ids"]).astype(np.int64)
    l_ids = np.asarray(inputs["l_ids"]).astype(np.int64)
    s_ids = np.asarray(inputs["s_ids"]).astype(np.int64)
    wproj = np.asarray(inputs["W_proj"], np.float32)
    bproj = np.asarray(inputs["b_proj"], np.float32)
    wmerge = np.asarray(inputs["W_merge"], np.float32)
    bmerge = np.asarray(inputs["b_merge"], np.float32)
    M = b_ids.shape[0]

    # pad + HWC layout: [B, HP, WP, C]
    fpadT = [
        np.ascontiguousarray(
            np.pad(f, ((0, 0), (0, 0), (2, 2), (2, 2))).transpose(0, 2, 3, 1))
        for f in (f0, f1)
    ]

    groups = []  # (positions_into_2M, ids, branch, b, half)
    for branch, ids in ((0, l_ids), (1, s_ids)):
        h = ids // WO
        for bb in range(B):
            for half in range(2):
                mask = (b_ids == bb) & ((h >= 30) if half else (h < 30))
                pos = np.nonzero(mask)[0] + branch * M
                groups.append((pos, ids[mask], branch, bb, half))

    maxcnt = max(len(g[1]) for g in groups)
    CAP = max(((maxcnt + 127) // 128) * 128, 128)

    in_maps = []
    for pos, ids, branch, bb, half in groups:
        rs = 120 if half else 0
        fm = fpadT[branch][bb, rs:rs + ROWS]               # [121, 324, 128]
        cf = (c0, c1)[branch][bb]                          # [4800, 256]

        idp = np.zeros(CAP, np.int64)
        idp[:len(ids)] = ids
        if len(ids) < CAP:
            idp[len(ids):] = 0 if half == 0 else 30 * WO
        h = idp // WO
        w = idp % WO
        # window-row gather blocks: row (4h - rs + r), col block w (4px units).
        # dma_gather places row j at [j%128, j//128]; order rows so item m
        # (chunk-local) lands on partition m with its 5 ki rows in slots 0-4:
        # position (within chunk) = ki*128 + m_local.
        blk = ((4 * h - rs)[:, None] + np.arange(5)[None, :]) * (WP // 4) + w[:, None]
        blk = blk.reshape(-1, GC, 5).transpose(0, 2, 1)   # [chunk, ki, m_local]
        fidx = _wrap16(blk.reshape(-1).astype(np.int16), CAP * 5 // 16)
        cidx = _wrap16(idp.astype(np.int16), CAP // 16)

        in_maps.append({
            "fmap": np.ascontiguousarray(fm).reshape(1, -1),
            "cfeat": np.ascontiguousarray(cf).reshape(1, -1),
            "fidx": fidx,
            "cidx": cidx,
            "wproj": wproj,
            "wmerge": wmerge,
            "bproj": bproj,
            "bmerge": bmerge,
        })
    return in_maps, groups, CAP, M


def _assemble(results, groups, M):
    full = np.empty((2 * M, 25, 128), np.float32)
    for (pos, ids, *_), res in zip(groups, results):
        og = res["out"].reshape(128, -1, 25).transpose(1, 2, 0)
        full[pos] = og[:len(pos)]
    return full[:M], full[M:]


def _install_ntff_shim():
    """This image lacks ``antenv.axon_hooks``; recreate it so bass_utils'
    trace path can drive NTFF profiling via the axon PJRT .so."""
    import sys, types
    if "antenv.axon_hooks" in sys.modules:
        return
    import antenv  # noqa: F401
    mod = types.ModuleType("antenv.axon_hooks")
    mod._hook = None
    mod.set_axon_ntff_profile_hook = lambda h: setattr(mod, "_hook", h)
    mod.get_axon_ntff_profile_hook = lambda: mod._hook
    sys.modules["antenv.axon_hooks"] = mod
    try:
        from trn_agent_boot.trn_boot import _ntff_profile_via_ctypes
        mod._hook = _ntff_profile_via_ctypes("/opt/axon/libaxon_pjrt.so")
    except Exception:
        pass


def kernel(**inputs):
    from concourse import bass_utils

    in_maps, groups, CAP, M = _host_prep(inputs)
    nc = _build_program(CAP)

    if os.environ.get("TRNK_SIM"):
        from concourse.bass_interp import CoreSim
        results = []
        ncore = int(os.environ.get("TRNK_SIM_CORES", "8"))
        for c in range(8):
            if c < ncore:
                sim = CoreSim(nc, trace=False)
                for name, val in in_maps[c].items():
                    sim.tensor(name)[:] = val
                sim.simulate()
                results.append({"out": np.array(sim.tensor("out"))})
            else:
                results.append({"out": np.zeros(CAP * 3200, np.float32)})
        return _assemble(results, groups, M)

    trace = bool(os.environ.get("TRNK_TRACE"))
    kw = {}
    if trace:
        _install_ntff_shim()
        kw = dict(trace=True, trace_cores=list(range(8)))
    res = bass_utils.run_bass_kernel_spmd(nc, in_maps, core_ids=list(range(8)), **kw)
    if trace and res.exec_time_ns is not None:
        kernel.last_exec_time_ns = res.exec_time_ns
        kernel.last_mean_exec_time_ns = res.mean_exec_time_ns
        if res.instructions_and_trace:
            kernel.last_trace_path = res.instructions_and_trace[1]
    return _assemble(res.results, groups, M)


kernel.last_exec_time_ns = None
kernel.last_mean_exec_time_ns = None
kernel.last_trace_path = None
